# revision 20
# baseline (speedup 1.0000x reference)
"""LIF spike (leaky integrate-and-fire with hard reset) Trainium2 kernel.

x: [B=32, T=16, C=128, H=32, W=32] f32  ->  spikes, same shape.
Per element (b,c,h,w), sequential over t:
    v = mem*TAU + x_t ; s = (v >= TH) ; mem = v * (v < TH)

Sharding: batch dim B=32 split across 8 NeuronCores (4 per core), pure
data-parallel SPMD (no collectives).

Per-core pipeline (v7): all 4 local b's form one [C=128, 4*H*W=4096] tile.
The whole per-timestep recurrence runs as ONE fused custom-DVE op by
carrying the PRE-reset potential v as state:
    v_t = select(v_{t-1} < TH, v_{t-1}, 0) * TAU + x_t
(one DVE pass instead of two scalar_tensor_tensor passes -- the DVE was
the measured bottleneck).  ACT computes sig_t = Sign(v_t - TH) into fp8
(exact; -1 below threshold, +0/+1 at/above).

Output compression: for timestep chunks 0-2 (t=0..11) the idle
TensorEngine packs 4 sign-steps into one byte via diagonal-weight
matmuls accumulated in PSUM: packed = sum_th sig_th * 2^th, an exact
small integer in [-15, 15] representable in fp8e4m3.  The host decodes
the signed base-2 digits (generic sig in {-1,+1}; sig=0 needs v==TH
exactly, probability ~0).  The last chunk is stored as raw per-step
sign bytes so the kernel tail stays one small store.  HBM write traffic
drops from 16 B to 5.5 B per element-row (1.5 MB + 2 MB per core).

Input loads alternate the two HWDGE rings (Sync + Scalar); the ~410
GB/s SDMA pool is shared round-robin across the active queues.
Scalar-ring loads are emitted ahead of their consumer, before that
iteration's sign op, so they issue early instead of queueing behind
ACT compute (head-of-line starvation).  The first two tiles are split
across both rings to halve the time-to-first-compute.
"""

import sys

import ml_dtypes
import numpy as np

for _p in ("/opt/trn_rl_repo",):
    if _p not in sys.path:
        sys.path.insert(0, _p)

import concourse.bacc as bacc
import concourse.bass as bass
import concourse.mybir as mybir
from concourse.bass_utils import run_bass_kernel_spmd
from concourse.tile import TileContext

B, T, C, H, W = 32, 16, 128, 32, 32
HW = H * W
N_CORES = 8
BL = B // N_CORES  # 4 batches per core
GF = BL * HW  # 4096: all local batches in one tile's free dim
TCH = 4  # timesteps per pack/store chunk
NPACK = 3  # chunks 0..2 are PE-packed; chunk 3 stored raw
TAU = 0.25
TH = 0.5
MM_N = 512  # one PSUM bank of f32 per matmul

_nc_cache = None
_lif_op_cache = None


def _get_lif_op():
    """Define + register the fused LIF-step custom DVE op.

    out = select(in0 < s1, in0, 0) * s0 + in1
        = reset(v_prev) * TAU + x_t
    """
    global _lif_op_cache
    if _lif_op_cache is not None:
        return _lif_op_cache
    import concourse.dve_ops as dve_ops_mod
    from concourse.dve_ops import DveOp
    from concourse.dve_spec import C0, C1, Spec, Src0, Src1, Zero, lower, select
    from concourse.dve_uop import DveOpSpec

    name = "LIF_STEP_ANT"
    for op in dve_ops_mod.OPS:
        if op.name == name:
            _lif_op_cache = op
            return op

    r = select(Src0 < C1, Src0, Zero)
    body = r * C0 + Src1

    def _ref(in0, in1, s0, s1, imm2):
        m = np.where(in0 < s1, in0, 0.0).astype(np.float32)
        return (m * np.float32(s0) + in1).astype(np.float32)

    spec = Spec(body=body, reference=_ref)
    shas = {
        ver: DveOpSpec(name=name, uops=lower(spec, ver=ver), rd1_en=True).sha(ver)
        for ver in ("v3", "v4")
    }
    op = DveOp(name, spec, subdim=False, uops_sha=shas)
    dve_ops_mod.OPS.append(op)
    dve_ops_mod._SUB_OPCODE_FOR_NAME[name] = (
        max(dve_ops_mod._SUB_OPCODE_FOR_NAME.values()) + 1
    )
    dve_ops_mod.CUSTOM_DVE_SPECS[name] = spec
    _lif_op_cache = op
    return op


def _weights_host() -> np.ndarray:
    """[C, TCH*C] fp8 diag blocks: W[c, th*C + c] = 2**th (exact in e4m3)."""
    w = np.zeros((C, TCH * C), dtype=np.float32)
    for th in range(TCH):
        w[np.arange(C), th * C + np.arange(C)] = float(1 << th)
    return w.astype(ml_dtypes.float8_e4m3fn)


def _build_nc():
    lif_op = _get_lif_op()
    nc = bacc.Bacc(
        "TRN2", target_bir_lowering=False, debug=False, num_devices=N_CORES
    )
    x = nc.dram_tensor("x", [BL, T, C, HW], mybir.dt.float32, kind="ExternalInput")
    w = nc.dram_tensor("w", [C, TCH * C], mybir.dt.float8e4, kind="ExternalInput")
    # packed sign digits for chunks 0..NPACK-1
    p8 = nc.dram_tensor(
        "p8", [C, NPACK * GF], mybir.dt.float8e4, kind="ExternalOutput"
    )
    # raw per-step signs for the last chunk
    s3 = nc.dram_tensor(
        "s3", [C, BL, TCH * HW], mybir.dt.float8e4, kind="ExternalOutput"
    )

    with TileContext(nc) as tc:
        with (
            tc.tile_pool(name="const", bufs=1) as cp,
            tc.tile_pool(name="mem", bufs=3) as mp,
            tc.tile_pool(name="xin", bufs=7) as xp,
            tc.tile_pool(name="sgn", bufs=6) as gp,
            tc.tile_pool(name="pk", bufs=3) as kp,
            tc.psum_pool(name="acc", bufs=1) as pp,
        ):
            neg_th = cp.tile([C, 1], mybir.dt.float32, tag="neg_th")
            nc.vector.memset(neg_th[:], -TH)

            xts = [None] * T

            def load_x(t, split=False):
                xt = xp.tile([C, BL, HW], mybir.dt.float32, tag="x")
                if split:
                    nc.sync.dma_start(
                        out=xt[:, :2], in_=x[:2, t].rearrange("b c f -> c b f")
                    )
                    nc.scalar.dma_start(
                        out=xt[:, 2:], in_=x[2:, t].rearrange("b c f -> c b f")
                    )
                else:
                    # t=15 rides Sync so the ring byte-counts balance
                    # (Scalar also carries the ~3.5 MB of stores)
                    on_sync = t % 2 == 0 or t == T - 1
                    dma_eng = nc.sync if on_sync else nc.scalar
                    dma_eng.dma_start(
                        out=xt[:], in_=x[:, t].rearrange("b c f -> c b f")
                    )
                xts[t] = xt

            load_x(0, split=True)
            load_x(1, split=True)
            load_x(2)
            load_x(3)
            wt = cp.tile([C, TCH * C], mybir.dt.float8e4, tag="w")
            nc.sync.dma_start(out=wt[:], in_=w[:, :])

            v_prev = None
            psum = None
            deferred_stores = []
            for t in range(T):
                th = t % TCH
                chunk = t // TCH
                if t + 4 < T:
                    load_x(t + 4)
                v = mp.tile([C, GF], mybir.dt.float32, tag="mem")
                xf = xts[t][:].rearrange("c b f -> c (b f)")
                xts[t] = None
                HF = GF // 2
                if t == 0:
                    # v = x (select(..)*0 + x); split into ring-halves so
                    # each half starts as soon as its ring delivers
                    for sl in (slice(0, HF), slice(HF, GF)):
                        nc.vector._custom_dve(
                            lif_op,
                            out=v[:, sl],
                            in0=xf[:, sl],
                            in1=xf[:, sl],
                            s0=0.0,
                            s1=TH,
                        )
                else:
                    # v = select(v_prev < TH, v_prev, 0)*TAU + x
                    nc.vector._custom_dve(
                        lif_op, out=v[:], in0=v_prev[:], in1=xf, s0=TAU, s1=TH
                    )
                v_prev = v
                # sig = Sign(v - TH): -1 below threshold, 0/+1 at/above
                sg = gp.tile([C, BL, HW], mybir.dt.float8e4, tag="sg")
                if t in (0, T - 1):
                    # halved so the first signs start earlier / the last
                    # stores launch earlier
                    for h in range(2):
                        nc.scalar.sign(
                            out=sg[:, 2 * h : 2 * h + 2],
                            in_=v[:, h * HF : (h + 1) * HF].rearrange(
                                "c (b f) -> c b f", b=2
                            ),
                            bias=neg_th[:],
                        )
                        if t == T - 1:
                            eng = nc.sync if h == 0 else nc.scalar
                            deferred_stores.append(
                                (
                                    eng,
                                    s3[
                                        :,
                                        2 * h : 2 * h + 2,
                                        th * HW : (th + 1) * HW,
                                    ],
                                    sg[:, 2 * h : 2 * h + 2],
                                )
                            )
                else:
                    nc.scalar.sign(
                        out=sg[:],
                        in_=v[:].rearrange("c (b f) -> c b f", b=BL),
                        bias=neg_th[:],
                    )
                if chunk < NPACK:
                    # pack: psum[:, j] += 2^th * sig   (diag-weight matmul)
                    if th == 0:
                        psum = pp.tile([C, GF], mybir.dt.float32, tag="acc")
                    sgf = sg[:].rearrange("c b f -> c (b f)")
                    for j in range(GF // MM_N):
                        nc.tensor.matmul(
                            psum[:, j * MM_N : (j + 1) * MM_N],
                            wt[:, th * C : (th + 1) * C],
                            sgf[:, j * MM_N : (j + 1) * MM_N],
                            start=(th == 0),
                            stop=(th == TCH - 1),
                        )
                    if th == TCH - 1:
                        pk = kp.tile([C, GF], mybir.dt.float8e4, tag="pk")
                        nc.scalar.copy(out=pk[:], in_=psum[:])
                        deferred_stores.append(
                            (nc.sync, p8[:, chunk * GF : (chunk + 1) * GF], pk[:])
                        )
                elif t != T - 1:
                    # last chunk: raw signs per-step; t=15 in halves above
                    deferred_stores.append(
                        (nc.scalar, s3[:, :, th * HW : (th + 1) * HW], sg[:])
                    )
            # All output stores are deferred behind the input loads: the
            # ~410 GB/s SDMA pool is saturated end-to-end, so a mid-run
            # store byte delays the last input tile one-for-one, while the
            # post-compute tail leaves the pool idle.  The logical-time
            # hint orders them last on their engine queues.
            with tc.tile_wait_until(0.5):
                for eng, dst, src in deferred_stores:
                    eng.dma_start(out=dst, in_=src)
    nc.compile()
    return nc


def _get_nc():
    global _nc_cache
    if _nc_cache is None:
        _nc_cache = _build_nc()
    return _nc_cache


def _ensure_ntff_hook():
    """Install the antenv.axon_hooks shim so trace=True works under axon.

    The agent image's antenv package lacks axon_hooks; build the same
    ctypes-based hook trn_agent_boot would have registered.
    """
    import types

    try:
        from antenv import axon_hooks  # noqa: F401

        return
    except ImportError:
        pass
    import antenv
    from trn_agent_boot.trn_boot import _ntff_profile_via_ctypes

    hook = _ntff_profile_via_ctypes("/opt/axon/libaxon_pjrt.so")
    mod = types.ModuleType("antenv.axon_hooks")
    holder = {"hook": hook}
    mod.set_axon_ntff_profile_hook = lambda h: holder.__setitem__("hook", h)
    mod.get_axon_ntff_profile_hook = lambda: holder["hook"]
    sys.modules["antenv.axon_hooks"] = mod
    antenv.axon_hooks = mod


def _digit_lut() -> np.ndarray:
    """[256, TCH] uint8 spike bits: fp8 byte -> signed base-2 digits -> spikes.

    packed = sum_th d_th * 2^th with d in {-1, 0, +1}; generically d is
    +-1 (all-odd sums decode uniquely via the sign of the remainder).
    d == 0 only when v == TH exactly; residual 0 decodes as all-spike
    (sign 0 => v >= TH => spike), matching that case.
    """
    vals = np.arange(256, dtype=np.uint8).view(ml_dtypes.float8_e4m3fn).astype(
        np.float32
    )
    lut = np.zeros((256, TCH), dtype=bool)
    for byte in range(256):
        r = float(vals[byte])
        if not np.isfinite(r):
            continue
        for th in range(TCH - 1, -1, -1):
            if r == 0.0:
                d = 0
            else:
                d = 1 if r > 0 else -1
            r -= d * (1 << th)
            lut[byte, th] = d >= 0  # sign >= 0  <=>  v >= TH  <=> spike
    return lut


_DIGIT_LUT = None


def kernel(x: np.ndarray, _trace: bool = False, **_unused):
    global _DIGIT_LUT
    assert x.shape == (B, T, C, H, W), x.shape
    if _trace:
        _ensure_ntff_hook()
    xr = np.ascontiguousarray(x, dtype=np.float32).reshape(B, T, C, HW)
    nc = _get_nc()
    wt = _weights_host()
    in_maps = [
        {"x": xr[i * BL : (i + 1) * BL], "w": wt} for i in range(N_CORES)
    ]
    res = run_bass_kernel_spmd(
        nc, in_maps, core_ids=list(range(N_CORES)), trace=_trace
    )
    if _DIGIT_LUT is None:
        _DIGIT_LUT = _digit_lut()
    outs = []
    for r in res.results:
        # chunks 0..2: packed digits
        praw = np.asarray(r["p8"]).view(np.uint8).reshape(C, NPACK, BL, HW)
        spk_p = _DIGIT_LUT[praw]  # [C, NPACK, BL, HW, TCH] bool
        spk_p = spk_p.transpose(2, 1, 4, 0, 3)  # -> [BL, NPACK, TCH, C, HW]
        spk_p = spk_p.reshape(BL, NPACK * TCH, C, HW)
        # chunk 3: raw signs, spike = sign bit clear
        sraw = np.asarray(r["s3"]).view(np.uint8).reshape(C, BL, TCH, HW)
        spk_r = (sraw < 0x80).transpose(1, 2, 0, 3)  # -> [BL, TCH, C, HW]
        outs.append(np.concatenate([spk_p, spk_r], axis=1))  # [BL, T, C, HW]
    out = np.concatenate(outs, axis=0)  # [B, T, C, HW] bool
    out = out.astype(np.float32).reshape(B, T, C, H, W)
    if _trace:
        kernel.last_results = res
    return out


# revision 25
# speedup vs baseline: 1.0443x; 1.0443x over previous
"""LIF spike (leaky integrate-and-fire with hard reset) Trainium2 kernel.

x: [B=32, T=16, C=128, H=32, W=32] f32  ->  spikes, same shape.
Per element (b,c,h,w), sequential over t:
    v = mem*TAU + x_t ; s = (v >= TH) ; mem = v * (v < TH)

Sharding: batch dim B=32 split across 8 NeuronCores (4 per core), pure
data-parallel SPMD (no collectives).

Per-core pipeline (v7): all 4 local b's form one [C=128, 4*H*W=4096] tile.
The whole per-timestep recurrence runs as ONE fused custom-DVE op by
carrying the PRE-reset potential v as state:
    v_t = select(v_{t-1} < TH, v_{t-1}, 0) * TAU + x_t
(one DVE pass instead of two scalar_tensor_tensor passes -- the DVE was
the measured bottleneck).  ACT computes sig_t = Sign(v_t - TH) into fp8
(exact; -1 below threshold, +0/+1 at/above).

Output compression: for timestep chunks 0-2 (t=0..11) the idle
TensorEngine packs 4 sign-steps into one byte via diagonal-weight
matmuls accumulated in PSUM: packed = sum_th sig_th * 2^th, an exact
small integer in [-15, 15] representable in fp8e4m3.  The host decodes
the signed base-2 digits (generic sig in {-1,+1}; sig=0 needs v==TH
exactly, probability ~0).  The last chunk is stored as raw per-step
sign bytes so the kernel tail stays one small store.  HBM write traffic
drops from 16 B to 5.5 B per element-row (1.5 MB + 2 MB per core).

Input loads alternate the two HWDGE rings (Sync + Scalar); the ~410
GB/s SDMA pool is shared round-robin across the active queues.
Scalar-ring loads are emitted ahead of their consumer, before that
iteration's sign op, so they issue early instead of queueing behind
ACT compute (head-of-line starvation).  The first two tiles are split
across both rings to halve the time-to-first-compute.
"""

import sys

import ml_dtypes
import numpy as np

for _p in ("/opt/trn_rl_repo",):
    if _p not in sys.path:
        sys.path.insert(0, _p)

import concourse.bacc as bacc
import concourse.bass as bass
import concourse.mybir as mybir
from concourse.bass_utils import run_bass_kernel_spmd
from concourse.tile import TileContext

B, T, C, H, W = 32, 16, 128, 32, 32
HW = H * W
N_CORES = 8
BL = B // N_CORES  # 4 batches per core
GF = BL * HW  # 4096: all local batches in one tile's free dim
TCH = 4  # timesteps per pack/store chunk
NPACK = 3  # chunks 0..2 are PE-packed; chunk 3 stored raw
TAU = 0.25
TH = 0.5
MM_N = 512  # one PSUM bank of f32 per matmul

_nc_cache = None
_lif_op_cache = None


def _get_lif_op():
    """Define + register the fused LIF-step custom DVE op.

    out = select(in0 < s1, in0, 0) * s0 + in1
        = reset(v_prev) * TAU + x_t
    """
    global _lif_op_cache
    if _lif_op_cache is not None:
        return _lif_op_cache
    import concourse.dve_ops as dve_ops_mod
    from concourse.dve_ops import DveOp
    from concourse.dve_spec import C0, C1, Spec, Src0, Src1, Zero, lower, select
    from concourse.dve_uop import DveOpSpec

    name = "LIF_STEP_ANT"
    for op in dve_ops_mod.OPS:
        if op.name == name:
            _lif_op_cache = op
            return op

    r = select(Src0 < C1, Src0, Zero)
    body = r * C0 + Src1

    def _ref(in0, in1, s0, s1, imm2):
        m = np.where(in0 < s1, in0, 0.0).astype(np.float32)
        return (m * np.float32(s0) + in1).astype(np.float32)

    spec = Spec(body=body, reference=_ref)
    shas = {
        ver: DveOpSpec(name=name, uops=lower(spec, ver=ver), rd1_en=True).sha(ver)
        for ver in ("v3", "v4")
    }
    op = DveOp(name, spec, subdim=False, uops_sha=shas)
    dve_ops_mod.OPS.append(op)
    dve_ops_mod._SUB_OPCODE_FOR_NAME[name] = (
        max(dve_ops_mod._SUB_OPCODE_FOR_NAME.values()) + 1
    )
    dve_ops_mod.CUSTOM_DVE_SPECS[name] = spec
    _lif_op_cache = op
    return op


def _weights_host() -> np.ndarray:
    """[C, TCH*C] fp8 diag blocks: W[c, th*C + c] = 2**th (exact in e4m3)."""
    w = np.zeros((C, TCH * C), dtype=np.float32)
    for th in range(TCH):
        w[np.arange(C), th * C + np.arange(C)] = float(1 << th)
    return w.astype(ml_dtypes.float8_e4m3fn)


def _build_nc():
    lif_op = _get_lif_op()
    nc = bacc.Bacc(
        "TRN2", target_bir_lowering=False, debug=False, num_devices=N_CORES
    )
    x = nc.dram_tensor("x", [BL, T, C, HW], mybir.dt.float32, kind="ExternalInput")
    w = nc.dram_tensor("w", [C, TCH * C], mybir.dt.float8e4, kind="ExternalInput")
    # packed sign digits for chunks 0..NPACK-1
    p8 = nc.dram_tensor(
        "p8", [C, NPACK * GF], mybir.dt.float8e4, kind="ExternalOutput"
    )
    # raw per-step signs for the last chunk
    s3 = nc.dram_tensor(
        "s3", [C, BL, TCH * HW], mybir.dt.float8e4, kind="ExternalOutput"
    )

    with TileContext(nc) as tc:
        with (
            tc.tile_pool(name="const", bufs=1) as cp,
            tc.tile_pool(name="mem", bufs=3) as mp,
            tc.tile_pool(name="xin", bufs=7) as xp,
            tc.tile_pool(name="sgn", bufs=3) as gp,
            tc.tile_pool(name="pk", bufs=2) as kp,
            tc.psum_pool(name="acc", bufs=1) as pp,
        ):
            neg_th = cp.tile([C, 1], mybir.dt.float32, tag="neg_th")
            nc.vector.memset(neg_th[:], -TH)

            xts = [None] * T

            def load_x(t, split=False):
                xt = xp.tile([C, BL, HW], mybir.dt.float32, tag="x")
                if split:
                    nc.sync.dma_start(
                        out=xt[:, :2], in_=x[:2, t].rearrange("b c f -> c b f")
                    )
                    nc.scalar.dma_start(
                        out=xt[:, 2:], in_=x[2:, t].rearrange("b c f -> c b f")
                    )
                else:
                    # t=15 rides Sync so the ring byte-counts balance
                    # (Scalar also carries the ~3.5 MB of stores)
                    on_sync = t % 2 == 0 or t == T - 1
                    dma_eng = nc.sync if on_sync else nc.scalar
                    dma_eng.dma_start(
                        out=xt[:], in_=x[:, t].rearrange("b c f -> c b f")
                    )
                xts[t] = xt

            load_x(0, split=True)
            load_x(1, split=True)
            load_x(2)
            load_x(3)
            wt = cp.tile([C, TCH * C], mybir.dt.float8e4, tag="w")
            nc.sync.dma_start(out=wt[:], in_=w[:, :])

            v_prev = None
            psum = None
            for t in range(T):
                th = t % TCH
                chunk = t // TCH
                if t + 4 < T:
                    load_x(t + 4)
                v = mp.tile([C, GF], mybir.dt.float32, tag="mem")
                xf = xts[t][:].rearrange("c b f -> c (b f)")
                xts[t] = None
                HF = GF // 2
                if t == 0:
                    # v = x (select(..)*0 + x); split into ring-halves so
                    # each half starts as soon as its ring delivers
                    for sl in (slice(0, HF), slice(HF, GF)):
                        nc.vector._custom_dve(
                            lif_op,
                            out=v[:, sl],
                            in0=xf[:, sl],
                            in1=xf[:, sl],
                            s0=0.0,
                            s1=TH,
                        )
                else:
                    # v = select(v_prev < TH, v_prev, 0)*TAU + x
                    nc.vector._custom_dve(
                        lif_op, out=v[:], in0=v_prev[:], in1=xf, s0=TAU, s1=TH
                    )
                v_prev = v
                # sig = Sign(v - TH): -1 below threshold, 0/+1 at/above
                sg = gp.tile([C, BL, HW], mybir.dt.float8e4, tag="sg")
                if t in (0, T - 1):
                    # halved so the first signs start earlier / the last
                    # stores launch earlier
                    for h in range(2):
                        nc.scalar.sign(
                            out=sg[:, 2 * h : 2 * h + 2],
                            in_=v[:, h * HF : (h + 1) * HF].rearrange(
                                "c (b f) -> c b f", b=2
                            ),
                            bias=neg_th[:],
                        )
                        if t == T - 1:
                            eng = nc.sync if h == 0 else nc.scalar
                            eng.dma_start(
                                out=s3[
                                    :, 2 * h : 2 * h + 2, th * HW : (th + 1) * HW
                                ],
                                in_=sg[:, 2 * h : 2 * h + 2],
                            )
                else:
                    nc.scalar.sign(
                        out=sg[:],
                        in_=v[:].rearrange("c (b f) -> c b f", b=BL),
                        bias=neg_th[:],
                    )
                if chunk < NPACK:
                    # pack: psum[:, j] += 2^th * sig   (diag-weight matmul)
                    if th == 0:
                        psum = pp.tile([C, GF], mybir.dt.float32, tag="acc")
                    sgf = sg[:].rearrange("c b f -> c (b f)")
                    for j in range(GF // MM_N):
                        nc.tensor.matmul(
                            psum[:, j * MM_N : (j + 1) * MM_N],
                            wt[:, th * C : (th + 1) * C],
                            sgf[:, j * MM_N : (j + 1) * MM_N],
                            start=(th == 0),
                            stop=(th == TCH - 1),
                        )
                    if th == TCH - 1:
                        pk = kp.tile([C, GF], mybir.dt.float8e4, tag="pk")
                        nc.scalar.copy(out=pk[:], in_=psum[:])
                        nc.scalar.dma_start(
                            out=p8[:, chunk * GF : (chunk + 1) * GF], in_=pk[:]
                        )
                elif t != T - 1:
                    # last chunk: store raw signs per-step (short tail);
                    # t=15 is stored in halves above
                    nc.scalar.dma_start(
                        out=s3[:, :, th * HW : (th + 1) * HW], in_=sg[:]
                    )
    nc.compile()
    return nc


def _get_nc():
    global _nc_cache
    if _nc_cache is None:
        _nc_cache = _build_nc()
    return _nc_cache


def _ensure_ntff_hook():
    """Install the antenv.axon_hooks shim so trace=True works under axon.

    The agent image's antenv package lacks axon_hooks; build the same
    ctypes-based hook trn_agent_boot would have registered.
    """
    import types

    try:
        from antenv import axon_hooks  # noqa: F401

        return
    except ImportError:
        pass
    import antenv
    from trn_agent_boot.trn_boot import _ntff_profile_via_ctypes

    hook = _ntff_profile_via_ctypes("/opt/axon/libaxon_pjrt.so")
    mod = types.ModuleType("antenv.axon_hooks")
    holder = {"hook": hook}
    mod.set_axon_ntff_profile_hook = lambda h: holder.__setitem__("hook", h)
    mod.get_axon_ntff_profile_hook = lambda: holder["hook"]
    sys.modules["antenv.axon_hooks"] = mod
    antenv.axon_hooks = mod


def _digit_lut() -> np.ndarray:
    """[256, TCH] uint8 spike bits: fp8 byte -> signed base-2 digits -> spikes.

    packed = sum_th d_th * 2^th with d in {-1, 0, +1}; generically d is
    +-1 (all-odd sums decode uniquely via the sign of the remainder).
    d == 0 only when v == TH exactly; residual 0 decodes as all-spike
    (sign 0 => v >= TH => spike), matching that case.
    """
    vals = np.arange(256, dtype=np.uint8).view(ml_dtypes.float8_e4m3fn).astype(
        np.float32
    )
    lut = np.zeros((256, TCH), dtype=bool)
    for byte in range(256):
        r = float(vals[byte])
        if not np.isfinite(r):
            continue
        for th in range(TCH - 1, -1, -1):
            if r == 0.0:
                d = 0
            else:
                d = 1 if r > 0 else -1
            r -= d * (1 << th)
            lut[byte, th] = d >= 0  # sign >= 0  <=>  v >= TH  <=> spike
    return lut


_DIGIT_LUT = None


def kernel(x: np.ndarray, _trace: bool = False, **_unused):
    global _DIGIT_LUT
    assert x.shape == (B, T, C, H, W), x.shape
    if _trace:
        _ensure_ntff_hook()
    xr = np.ascontiguousarray(x, dtype=np.float32).reshape(B, T, C, HW)
    nc = _get_nc()
    wt = _weights_host()
    in_maps = [
        {"x": xr[i * BL : (i + 1) * BL], "w": wt} for i in range(N_CORES)
    ]
    res = run_bass_kernel_spmd(
        nc, in_maps, core_ids=list(range(N_CORES)), trace=_trace
    )
    if _DIGIT_LUT is None:
        _DIGIT_LUT = _digit_lut()
    outs = []
    for r in res.results:
        # chunks 0..2: packed digits
        praw = np.asarray(r["p8"]).view(np.uint8).reshape(C, NPACK, BL, HW)
        spk_p = _DIGIT_LUT[praw]  # [C, NPACK, BL, HW, TCH] bool
        spk_p = spk_p.transpose(2, 1, 4, 0, 3)  # -> [BL, NPACK, TCH, C, HW]
        spk_p = spk_p.reshape(BL, NPACK * TCH, C, HW)
        # chunk 3: raw signs, spike = sign bit clear
        sraw = np.asarray(r["s3"]).view(np.uint8).reshape(C, BL, TCH, HW)
        spk_r = (sraw < 0x80).transpose(1, 2, 0, 3)  # -> [BL, TCH, C, HW]
        outs.append(np.concatenate([spk_p, spk_r], axis=1))  # [BL, T, C, HW]
    out = np.concatenate(outs, axis=0)  # [B, T, C, HW] bool
    out = out.astype(np.float32).reshape(B, T, C, H, W)
    if _trace:
        kernel.last_results = res
    return out
